# revision 14
# baseline (speedup 1.0000x reference)
"""Block-wise embedding lookup on 8 Trainium2 NeuronCores.

Strategy: data-parallel over tokens. Each of the 8 cores gets 8192 of the
65536 tokens; the concatenated embedding table (100000 x 512 f32) is
replicated to every core. The row index per token,
  gidx = offsets[block_assign[src]] + local_assign[src],
is tiny integer work (0.8 MB of lookups) done on the host during input
sharding; the memory-bound 128 MB row gather runs on the device.

Per core the device pipeline is 64 groups of 128 tokens:
  indirect-DMA gather big[gidx[group]] -> SBUF [128, 512]   (SWDGE, gpsimd)
  direct DMA         SBUF -> out[group rows]                (HWDGE, sync)
with an 8-deep SBUF buffer/semaphore ring so gathers, writes and their
completions overlap. TRN2's indirect DMA gathers one row per partition
per instruction (128 rows/DMA), so 64 gathers cover the 8192 tokens.

Raw bass (no TileContext): this toolchain accepts only one sync-wait
command per instruction, so all synchronization is standalone wait_ge
instructions and every in-flight DMA has its own semaphore slot.
"""

import numpy as np

N_CORES = 8
B, S, DIM, VOCAB = 32, 2048, 512, 100000
TOK = B * S                 # 65536 tokens total
TPC = TOK // N_CORES        # 8192 tokens per core
P = 128                     # SBUF partitions
NG = TPC // P               # 64 token groups per core
NB = 8                      # SBUF buffer ring / semaphore ring depth
BLOCK_OFFSETS = np.array([0, 50000, 80000, 95000], dtype=np.int32)

_CACHE = {}


def _build_nc():
    from contextlib import ExitStack
    from concourse import bass, mybir

    nc = bass.Bass()
    gidx_d = nc.declare_dram_parameter("gidx", [P, NG], mybir.dt.int32, isOutput=False)
    big = nc.declare_dram_parameter(
        "big", [VOCAB, DIM], mybir.dt.float32, isOutput=False
    )
    out = nc.declare_dram_parameter("out", [TPC, DIM], mybir.dt.float32, isOutput=True)

    with ExitStack() as ctx:
        block = ctx.enter_context(nc.Block())
        s0 = ctx.enter_context(nc.semaphore("s0"))
        sem_g = [ctx.enter_context(nc.semaphore(f"sg{i}")) for i in range(NB)]
        sem_w = [ctx.enter_context(nc.semaphore(f"sw{i}")) for i in range(NB)]
        gidx_t = ctx.enter_context(nc.sbuf_tensor("gidx_t", [P, NG], mybir.dt.int32))
        g = [
            ctx.enter_context(nc.sbuf_tensor(f"g{i}", [P, DIM], mybir.dt.float32))
            for i in range(NB)
        ]

        @block.sync
        def _(sync):
            sync.dma_start(out=gidx_t[:], in_=gidx_d[:]).then_inc(s0, 16)
            for c in range(NG):
                sync.wait_ge(sem_g[c % NB], 16 * (c // NB + 1))
                sync.dma_start(
                    out=out[c * P : (c + 1) * P, :], in_=g[c % NB][:]
                ).then_inc(sem_w[c % NB], 16)
            for i in range(NB):
                sync.wait_ge(sem_w[i], 16 * len(range(i, NG, NB)))

        @block.gpsimd
        def _(gpsimd):
            gpsimd.wait_ge(s0, 16)
            for c in range(NG):
                if c >= NB:
                    # buffer reuse: wait until the write of group c-NB drained
                    gpsimd.wait_ge(sem_w[c % NB], 16 * (c // NB))
                gpsimd.indirect_dma_start(
                    out=g[c % NB][:],
                    out_offset=None,
                    in_=big[:],
                    in_offset=bass.IndirectOffsetOnAxis(
                        ap=gidx_t[:, c : c + 1], axis=0
                    ),
                ).then_inc(sem_g[c % NB], 16)

    return nc


def _get_nc():
    if "nc" not in _CACHE:
        _CACHE["nc"] = _build_nc()
    return _CACHE["nc"]


def prepare_in_maps(src, block_assign, local_assign, table0, table1, table2, table3):
    big = np.ascontiguousarray(
        np.concatenate(
            [np.asarray(t, dtype=np.float32) for t in (table0, table1, table2, table3)],
            axis=0,
        )
    )
    assert big.shape == (VOCAB, DIM)
    ba = np.asarray(block_assign, np.int32).reshape(-1)
    la = np.asarray(local_assign, np.int32).reshape(-1)
    src_flat = np.asarray(src, np.int32).reshape(-1)
    gidx = BLOCK_OFFSETS[ba[src_flat]] + la[src_flat]  # [TOK]
    in_maps = []
    for k in range(N_CORES):
        # group c = tokens [c*128, (c+1)*128); gidx_d[p, c] = gidx[c*128+p]
        shard = (
            gidx[k * TPC : (k + 1) * TPC].reshape(NG, P).T.astype(np.int32).copy()
        )
        in_maps.append({"gidx": shard, "big": big})
    return in_maps


def assemble_output(results):
    parts = [np.asarray(r["out"]) for r in results]
    return np.concatenate(parts, axis=0).reshape(B, S, DIM)


def kernel(src, block_assign, local_assign, table0, table1, table2, table3):
    from concourse.bass_utils import run_bass_kernel_spmd

    nc = _get_nc()
    in_maps = prepare_in_maps(
        src, block_assign, local_assign, table0, table1, table2, table3
    )
    res = run_bass_kernel_spmd(nc, in_maps, list(range(N_CORES)))
    return assemble_output(res.results)


# revision 15
# speedup vs baseline: 1.1188x; 1.1188x over previous
"""Block-wise embedding lookup on 8 Trainium2 NeuronCores.

Strategy: data-parallel over tokens. Each of the 8 cores gets 8192 of the
65536 tokens; the concatenated embedding table (100000 x 512 f32) is
replicated to every core. The row index per token,
  gidx = offsets[block_assign[src]] + local_assign[src],
is tiny integer work (0.8 MB of lookups) done on the host during input
sharding; the memory-bound 128 MB row gather runs on the device.

Per core the device pipeline is 64 groups of 128 tokens:
  indirect-DMA gather big[gidx[group]] -> SBUF [128, 512]   (SWDGE, gpsimd)
  direct DMA         SBUF -> out[group rows]                (HWDGE, sync)
with an 8-deep SBUF buffer/semaphore ring so gathers, writes and their
completions overlap. TRN2's indirect DMA gathers one row per partition
per instruction (128 rows/DMA), so 64 gathers cover the 8192 tokens.

Raw bass (no TileContext): this toolchain accepts only one sync-wait
command per instruction, so all synchronization is standalone wait_ge
instructions and every in-flight DMA has its own semaphore slot.
"""

import numpy as np

N_CORES = 8
B, S, DIM, VOCAB = 32, 2048, 512, 100000
TOK = B * S                 # 65536 tokens total
TPC = TOK // N_CORES        # 8192 tokens per core
P = 128                     # SBUF partitions
NG = TPC // P               # 64 token groups per core
NB = 32                     # SBUF buffer ring / semaphore ring depth
BLOCK_OFFSETS = np.array([0, 50000, 80000, 95000], dtype=np.int32)

_CACHE = {}


def _build_nc():
    from contextlib import ExitStack
    from concourse import bass, mybir

    nc = bass.Bass()
    gidx_d = nc.declare_dram_parameter("gidx", [P, NG], mybir.dt.int32, isOutput=False)
    big = nc.declare_dram_parameter(
        "big", [VOCAB, DIM], mybir.dt.float32, isOutput=False
    )
    out = nc.declare_dram_parameter("out", [TPC, DIM], mybir.dt.float32, isOutput=True)

    with ExitStack() as ctx:
        block = ctx.enter_context(nc.Block())
        s0 = ctx.enter_context(nc.semaphore("s0"))
        sem_g = [ctx.enter_context(nc.semaphore(f"sg{i}")) for i in range(NB)]
        sem_w = [ctx.enter_context(nc.semaphore(f"sw{i}")) for i in range(NB)]
        gidx_t = ctx.enter_context(nc.sbuf_tensor("gidx_t", [P, NG], mybir.dt.int32))
        g = [
            ctx.enter_context(nc.sbuf_tensor(f"g{i}", [P, DIM], mybir.dt.float32))
            for i in range(NB)
        ]

        @block.sync
        def _(sync):
            sync.dma_start(out=gidx_t[:], in_=gidx_d[:]).then_inc(s0, 16)
            for c in range(NG):
                sync.wait_ge(sem_g[c % NB], 16 * (c // NB + 1))
                sync.dma_start(
                    out=out[c * P : (c + 1) * P, :], in_=g[c % NB][:]
                ).then_inc(sem_w[c % NB], 16)
            for i in range(NB):
                sync.wait_ge(sem_w[i], 16 * len(range(i, NG, NB)))

        @block.gpsimd
        def _(gpsimd):
            gpsimd.wait_ge(s0, 16)
            for c in range(NG):
                if c >= NB:
                    # buffer reuse: wait until the write of group c-NB drained
                    gpsimd.wait_ge(sem_w[c % NB], 16 * (c // NB))
                gpsimd.indirect_dma_start(
                    out=g[c % NB][:],
                    out_offset=None,
                    in_=big[:],
                    in_offset=bass.IndirectOffsetOnAxis(
                        ap=gidx_t[:, c : c + 1], axis=0
                    ),
                ).then_inc(sem_g[c % NB], 16)

    return nc


def _get_nc():
    if "nc" not in _CACHE:
        _CACHE["nc"] = _build_nc()
    return _CACHE["nc"]


def prepare_in_maps(src, block_assign, local_assign, table0, table1, table2, table3):
    big = np.ascontiguousarray(
        np.concatenate(
            [np.asarray(t, dtype=np.float32) for t in (table0, table1, table2, table3)],
            axis=0,
        )
    )
    assert big.shape == (VOCAB, DIM)
    ba = np.asarray(block_assign, np.int32).reshape(-1)
    la = np.asarray(local_assign, np.int32).reshape(-1)
    src_flat = np.asarray(src, np.int32).reshape(-1)
    gidx = BLOCK_OFFSETS[ba[src_flat]] + la[src_flat]  # [TOK]
    in_maps = []
    for k in range(N_CORES):
        # group c = tokens [c*128, (c+1)*128); gidx_d[p, c] = gidx[c*128+p]
        shard = (
            gidx[k * TPC : (k + 1) * TPC].reshape(NG, P).T.astype(np.int32).copy()
        )
        in_maps.append({"gidx": shard, "big": big})
    return in_maps


def assemble_output(results):
    parts = [np.asarray(r["out"]) for r in results]
    return np.concatenate(parts, axis=0).reshape(B, S, DIM)


def kernel(src, block_assign, local_assign, table0, table1, table2, table3):
    from concourse.bass_utils import run_bass_kernel_spmd

    nc = _get_nc()
    in_maps = prepare_in_maps(
        src, block_assign, local_assign, table0, table1, table2, table3
    )
    res = run_bass_kernel_spmd(nc, in_maps, list(range(N_CORES)))
    return assemble_output(res.results)


# revision 16
# speedup vs baseline: 1.1366x; 1.0159x over previous
"""Block-wise embedding lookup on 8 Trainium2 NeuronCores.

Strategy: data-parallel over tokens. Each of the 8 cores gets 8192 of the
65536 tokens; the concatenated embedding table (100000 x 512 f32) is
replicated to every core. The row index per token,
  gidx = offsets[block_assign[src]] + local_assign[src],
is tiny integer work (0.8 MB of lookups) done on the host during input
sharding; the memory-bound 128 MB row gather runs on the device.

Per core the device pipeline is 64 groups of 128 tokens:
  indirect-DMA gather big[gidx[group]] -> SBUF [128, 512]   (SWDGE, gpsimd)
  direct DMA         SBUF -> out[group rows]                (HWDGE, sync)
with an 8-deep SBUF buffer/semaphore ring so gathers, writes and their
completions overlap. TRN2's indirect DMA gathers one row per partition
per instruction (128 rows/DMA), so 64 gathers cover the 8192 tokens.

Raw bass (no TileContext): this toolchain accepts only one sync-wait
command per instruction, so all synchronization is standalone wait_ge
instructions and every in-flight DMA has its own semaphore slot.
"""

import numpy as np

N_CORES = 8
B, S, DIM, VOCAB = 32, 2048, 512, 100000
TOK = B * S                 # 65536 tokens total
TPC = TOK // N_CORES        # 8192 tokens per core
P = 128                     # SBUF partitions
NG = TPC // P               # 64 token groups per core
NB = 32                     # SBUF buffer ring / semaphore ring depth
BLOCK_OFFSETS = np.array([0, 50000, 80000, 95000], dtype=np.int32)

_CACHE = {}


def _build_nc():
    from contextlib import ExitStack
    from concourse import bass, mybir

    nc = bass.Bass()
    gidx_d = nc.declare_dram_parameter("gidx", [P, NG], mybir.dt.int32, isOutput=False)
    big = nc.declare_dram_parameter(
        "big", [VOCAB, DIM], mybir.dt.float32, isOutput=False
    )
    out = nc.declare_dram_parameter("out", [TPC, DIM], mybir.dt.float32, isOutput=True)

    with ExitStack() as ctx:
        block = ctx.enter_context(nc.Block())
        s0 = ctx.enter_context(nc.semaphore("s0"))
        sem_g = [ctx.enter_context(nc.semaphore(f"sg{i}")) for i in range(NB)]
        sem_w = [ctx.enter_context(nc.semaphore(f"sw{i}")) for i in range(NB)]
        gidx_t = ctx.enter_context(nc.sbuf_tensor("gidx_t", [P, NG], mybir.dt.int32))
        g = [
            ctx.enter_context(nc.sbuf_tensor(f"g{i}", [P, DIM], mybir.dt.float32))
            for i in range(NB)
        ]

        @block.sync
        def _(sync):
            sync.dma_start(out=gidx_t[:], in_=gidx_d[:]).then_inc(s0, 16)
            for c in range(NG):
                # wait for gather of group c, fused onto the write DMA
                sync.dma_start(
                    out=out[c * P : (c + 1) * P, :], in_=g[c % NB][:]
                )._wait_ge(sem_g[c % NB], 16 * (c // NB + 1)).then_inc(
                    sem_w[c % NB], 16
                )
            for i in range(NB):
                sync.wait_ge(sem_w[i], 16 * len(range(i, NG, NB)))

        @block.gpsimd
        def _(gpsimd):
            for c in range(NG):
                inst = gpsimd.indirect_dma_start(
                    out=g[c % NB][:],
                    out_offset=None,
                    in_=big[:],
                    in_offset=bass.IndirectOffsetOnAxis(
                        ap=gidx_t[:, c : c + 1], axis=0
                    ),
                ).then_inc(sem_g[c % NB], 16)
                if c == 0:
                    inst._wait_ge(s0, 16)  # gidx ids in SBUF
                elif c >= NB:
                    # buffer reuse: write of group c-NB must have drained
                    inst._wait_ge(sem_w[c % NB], 16 * (c // NB))

    return nc


def _get_nc():
    if "nc" not in _CACHE:
        _CACHE["nc"] = _build_nc()
    return _CACHE["nc"]


def prepare_in_maps(src, block_assign, local_assign, table0, table1, table2, table3):
    big = np.ascontiguousarray(
        np.concatenate(
            [np.asarray(t, dtype=np.float32) for t in (table0, table1, table2, table3)],
            axis=0,
        )
    )
    assert big.shape == (VOCAB, DIM)
    ba = np.asarray(block_assign, np.int32).reshape(-1)
    la = np.asarray(local_assign, np.int32).reshape(-1)
    src_flat = np.asarray(src, np.int32).reshape(-1)
    gidx = BLOCK_OFFSETS[ba[src_flat]] + la[src_flat]  # [TOK]
    in_maps = []
    for k in range(N_CORES):
        # group c = tokens [c*128, (c+1)*128); gidx_d[p, c] = gidx[c*128+p]
        shard = (
            gidx[k * TPC : (k + 1) * TPC].reshape(NG, P).T.astype(np.int32).copy()
        )
        in_maps.append({"gidx": shard, "big": big})
    return in_maps


def assemble_output(results):
    parts = [np.asarray(r["out"]) for r in results]
    return np.concatenate(parts, axis=0).reshape(B, S, DIM)


def kernel(src, block_assign, local_assign, table0, table1, table2, table3):
    from concourse.bass_utils import run_bass_kernel_spmd

    nc = _get_nc()
    in_maps = prepare_in_maps(
        src, block_assign, local_assign, table0, table1, table2, table3
    )
    res = run_bass_kernel_spmd(nc, in_maps, list(range(N_CORES)))
    return assemble_output(res.results)
